# revision 22
# baseline (speedup 1.0000x reference)
"""Fused QKV-projection + attention-softmax kernel for Trainium2 (8 NeuronCores).

Computes softmax((X @ Wq)(X @ Wk)^T / sqrt(dkv)) == the reference nn_Attention
attn_weights output [B=2, H=16, L=2048, L=2048] fp32.

Sharding: data-parallel over batch x tensor-parallel over heads.
core i -> batch i//4, heads [4*(i%4) .. 4*(i%4)+4). Each core:
  1. loads X[b] [2048, 1024] and PE-transposes it to XT [E, L] in SBUF
  2. projects Q^T/K^T per head directly in [feature, token] layout
     (W block as stationary operand), adds bias
  3. scores = Q^T.T @ K^T per 128-query tile into PSUM
  4. ACT: exp(0.125 * s) with fused per-row sum accumulation
  5. DVE: multiply by reciprocal of row sum, DMA out 1 MiB tiles
The V projection is dead code in the reference output and is skipped.
"""

from contextlib import ExitStack

import numpy as np

import concourse.bacc as bacc
import concourse.bass as bass
import concourse.mybir as mybir
import concourse.tile as tile
from concourse.bass import ts
from concourse.bass_utils import run_bass_kernel_spmd
from concourse.masks import make_identity

B, L, E = 2, 2048, 1024
H, DKV = 16, 64
HPC = 4          # heads per core
N_CORES = 8
P = 128
KT = E // P      # 8 contraction tiles for the projection
NQ = L // P      # 16 query tiles per head
NC512 = L // 512  # 4 512-wide chunks per row

F32 = mybir.dt.float32
BF16 = mybir.dt.bfloat16

# matmul-operand dtype: bf16 halves PE cycles (fp32 matmul is a 2-pass
# HI/LO decomposition on TRN2) and enables fast weight load. All
# accumulation (PSUM) and the softmax stay fp32.
MM_DT = BF16

# set by test.py to enable NTFF tracing; harness leaves it False
TRACE = False

_cached_nc = None


def _emit(tc, ctx):
    nc = tc.nc

    x_d = nc.dram_tensor("x", [E, L], MM_DT, kind="ExternalInput")  # X^T
    w_d = nc.dram_tensor("w", [E, HPC * P], MM_DT, kind="ExternalInput")
    b_d = nc.dram_tensor("bqk", [P, HPC], F32, kind="ExternalInput")
    out_d = nc.dram_tensor("out", [HPC, L, L], F32, kind="ExternalOutput")

    const = ctx.enter_context(tc.tile_pool(name="const", bufs=1))
    xtp = ctx.enter_context(tc.tile_pool(name="xt", bufs=1))
    qkp = ctx.enter_context(tc.tile_pool(name="qk", bufs=2))
    expp = ctx.enter_context(tc.tile_pool(name="exp", bufs=3))
    outp = ctx.enter_context(tc.tile_pool(name="outp", bufs=6))
    smalls = ctx.enter_context(tc.tile_pool(name="smalls", bufs=4))

    psum = ctx.enter_context(tc.tile_pool(name="psum", bufs=1, space="PSUM"))

    # W first (gates the first projection matmul), on the scalar queue.
    w_sb = const.tile([P, KT, HPC * P], MM_DT, tag="w")
    nc.scalar.dma_start(w_sb[:], w_d[:].rearrange("(kt p) f -> p kt f", p=P))

    # ---- load XT[:, et, tok] = X^T[et*128 + p, tok] (host pre-transposed,
    # so this is a plain fast DMA); chunked by token so projection chunk c
    # starts as soon as its quarter lands; chunks alternate between the two
    # HWDGE queues so they transfer in parallel.
    xt = xtp.tile([P, KT, L], MM_DT, tag="xt")
    for c in range(NC512):
        eng = nc.sync if c % 2 == 0 else nc.scalar
        eng.dma_start(
            xt[:, :, ts(c, 512)],
            x_d[:, ts(c, 512)].rearrange("(et p) t -> p et t", p=P),
        )

    bias_sb = const.tile([P, HPC], F32, tag="bias")
    nc.scalar.dma_start(bias_sb[:], b_d[:])

    # PE warm-up: dummy matmuls with no input deps keep the PE busy while
    # the first DMAs land, so HAM unthrottles (1.2 -> 2.4 GHz) before the
    # real projection starts.
    warm = const.tile([P, 512], MM_DT, tag="warm")
    nc.gpsimd.memset(warm[:], 0.0)
    for _ in range(18):
        pw = psum.tile([P, 512], F32, tag="scores", bufs=2)
        nc.tensor.matmul(pw[:], warm[:, 0:P], warm[:], start=True, stop=True)

    # w columns are host-reordered: block 2*pair   = [Q_h0 | Q_h1] (128 feats)
    #                               block 2*pair+1 = [K_h0 | K_h1]
    def proj_pair(pair):
        qt = qkp.tile([P, L], MM_DT, tag="qt")  # 0:64 = Q^T h0, 64:128 = Q^T h1
        kt_t = qkp.tile([P, L], MM_DT, tag="kt")
        # kt first: scores q-tile 0 needs ALL of kt but only chunk 0 of qt
        for dst, blk in ((kt_t, 2 * pair + 1), (qt, 2 * pair)):
            pp = psum.tile([P, L], F32, tag="scores", bufs=2)
            for c in range(NC512):
                for k in range(KT):
                    nc.tensor.matmul(
                        pp[:, ts(c, 512)],
                        w_sb[:, k, ts(blk, P)],
                        xt[:, k, ts(c, 512)],
                        start=(k == 0),
                        stop=(k == KT - 1),
                    )
                nc.vector.tensor_scalar_add(
                    dst[:, ts(c, 512)], pp[:, ts(c, 512)], bias_sb[:, blk : blk + 1]
                )
        return qt, kt_t

    def scores_head(qt, kt_t, h, off):
        for q in range(NQ):
            ps = psum.tile([P, L], F32, tag="scores", bufs=2)
            for c in range(NC512):
                nc.tensor.matmul(
                    ps[:, ts(c, 512)],
                    qt[off : off + DKV, ts(q, P)],
                    kt_t[off : off + DKV, ts(c, 512)],
                    start=True,
                    stop=True,
                )
            ex = expp.tile([P, L], F32, tag="exp")
            rsum = smalls.tile([P, 1], F32, tag="rsum")
            nc.scalar.activation(
                ex[:],
                ps[:],
                mybir.ActivationFunctionType.Exp,
                scale=1.0 / np.sqrt(DKV),
                accum_out=rsum[:],
            )
            rinv = smalls.tile([P, 1], F32, tag="rinv")
            nc.vector.reciprocal(rinv[:], rsum[:])
            ot = outp.tile([P, L], F32, tag="outp")
            nc.vector.tensor_scalar_mul(ot[:], ex[:], rinv[:])
            nc.sync.dma_start(out_d[h, ts(q, P), :], ot[:])

    # proj pair0 -> scores h0/h1 (output DMA starts early) -> proj pair1
    # -> scores h2/h3; proj pair1's PE work hides inside scores h0/h1.
    qt0, kt0 = proj_pair(0)
    scores_head(qt0, kt0, 0, 0)
    qt1, kt1 = proj_pair(1)
    scores_head(qt0, kt0, 1, DKV)
    scores_head(qt1, kt1, 2, 0)
    scores_head(qt1, kt1, 3, DKV)


def build():
    global _cached_nc
    if _cached_nc is not None:
        return _cached_nc
    nc = bacc.Bacc("TRN2", target_bir_lowering=False, debug=False)
    with tile.TileContext(nc) as tc, ExitStack() as ctx:
        _emit(tc, ctx)
    nc.compile()
    _cached_nc = nc
    return nc


def _shard_inputs(X, W_qkv, b_qkv):
    X = np.ascontiguousarray(np.asarray(X, dtype=np.float32))
    W = np.asarray(W_qkv, dtype=np.float32)
    bq = np.asarray(b_qkv, dtype=np.float32)
    in_maps = []
    for core in range(N_CORES):
        b = core // 4
        g = core % 4
        heads = list(range(g * HPC, (g + 1) * HPC))
        # per head h: W cols [h*3*DKV, h*3*DKV+DKV) = Q feats,
        #             [h*3*DKV+DKV, h*3*DKV+2*DKV) = K feats.
        # Reorder into per-pair stacked blocks: [Q_h0|Q_h1], [K_h0|K_h1], ...
        wq = [W[:, h * 3 * DKV : h * 3 * DKV + DKV] for h in heads]
        wk = [W[:, h * 3 * DKV + DKV : h * 3 * DKV + 2 * DKV] for h in heads]
        bqh = [bq[h * 3 * DKV : h * 3 * DKV + DKV] for h in heads]
        bkh = [bq[h * 3 * DKV + DKV : h * 3 * DKV + 2 * DKV] for h in heads]
        w_blocks, b_blocks = [], []
        for pair in range(HPC // 2):
            w_blocks += [wq[2 * pair], wq[2 * pair + 1]]
            w_blocks += [wk[2 * pair], wk[2 * pair + 1]]
            b_blocks += [np.concatenate([bqh[2 * pair], bqh[2 * pair + 1]])]
            b_blocks += [np.concatenate([bkh[2 * pair], bkh[2 * pair + 1]])]
        mm_np = mybir.dt.np(MM_DT)
        w_sel = np.concatenate(w_blocks, axis=1)
        b_sel = np.stack(b_blocks, axis=1)
        in_maps.append(
            {
                "x": np.ascontiguousarray(X[b].T).astype(mm_np),
                "w": np.ascontiguousarray(w_sel).astype(mm_np),
                "bqk": np.ascontiguousarray(b_sel),
            }
        )
    return in_maps


def kernel(X, W_qkv, b_qkv):
    nc = build()
    in_maps = _shard_inputs(X, W_qkv, b_qkv)
    res = run_bass_kernel_spmd(nc, in_maps, core_ids=list(range(N_CORES)), trace=TRACE)
    out = np.empty((B, H, L, L), dtype=np.float32)
    for core in range(N_CORES):
        b = core // 4
        g = core % 4
        out[b, g * HPC : (g + 1) * HPC] = res.results[core]["out"]
    kernel.last_results = res
    return out


# revision 25
# speedup vs baseline: 1.0066x; 1.0066x over previous
"""Fused QKV-projection + attention-softmax kernel for Trainium2 (8 NeuronCores).

Computes softmax((X @ Wq)(X @ Wk)^T / sqrt(dkv)) == the reference nn_Attention
attn_weights output [B=2, H=16, L=2048, L=2048] fp32.

Sharding: data-parallel over batch x tensor-parallel over heads.
core i -> batch i//4, heads [4*(i%4) .. 4*(i%4)+4). Each core:
  1. loads X[b] [2048, 1024] and PE-transposes it to XT [E, L] in SBUF
  2. projects Q^T/K^T per head directly in [feature, token] layout
     (W block as stationary operand), adds bias
  3. scores = Q^T.T @ K^T per 128-query tile into PSUM
  4. ACT: exp(0.125 * s) with fused per-row sum accumulation
  5. DVE: multiply by reciprocal of row sum, DMA out 1 MiB tiles
The V projection is dead code in the reference output and is skipped.
"""

from contextlib import ExitStack

import numpy as np

import concourse.bacc as bacc
import concourse.bass as bass
import concourse.mybir as mybir
import concourse.tile as tile
from concourse.bass import ts
from concourse.bass_utils import run_bass_kernel_spmd
from concourse.masks import make_identity

B, L, E = 2, 2048, 1024
H, DKV = 16, 64
HPC = 4          # heads per core
N_CORES = 8
P = 128
KT = E // P      # 8 contraction tiles for the projection
NQ = L // P      # 16 query tiles per head
NC512 = L // 512  # 4 512-wide chunks per row

F32 = mybir.dt.float32
BF16 = mybir.dt.bfloat16

# matmul-operand dtype: bf16 halves PE cycles (fp32 matmul is a 2-pass
# HI/LO decomposition on TRN2) and enables fast weight load. All
# accumulation (PSUM) and the softmax stay fp32.
MM_DT = BF16

# set by test.py to enable NTFF tracing; harness leaves it False
TRACE = False

_cached_nc = None


def _emit(tc, ctx):
    nc = tc.nc

    x_d = nc.dram_tensor("x", [E, L], MM_DT, kind="ExternalInput")  # X^T
    w_d = nc.dram_tensor("w", [E, HPC * P], MM_DT, kind="ExternalInput")
    b_d = nc.dram_tensor("bqk", [P, HPC], F32, kind="ExternalInput")
    out_d = nc.dram_tensor("out", [HPC, L, L], F32, kind="ExternalOutput")

    const = ctx.enter_context(tc.tile_pool(name="const", bufs=1))
    xtp = ctx.enter_context(tc.tile_pool(name="xt", bufs=1))
    qkp = ctx.enter_context(tc.tile_pool(name="qk", bufs=2))
    expp = ctx.enter_context(tc.tile_pool(name="exp", bufs=3))
    outp = ctx.enter_context(tc.tile_pool(name="outp", bufs=6))
    smalls = ctx.enter_context(tc.tile_pool(name="smalls", bufs=4))

    psum = ctx.enter_context(tc.tile_pool(name="psum", bufs=1, space="PSUM"))

    # W first (gates the first projection matmul), on the scalar queue.
    w_sb = const.tile([P, KT, HPC * P], MM_DT, tag="w")
    nc.scalar.dma_start(w_sb[:], w_d[:].rearrange("(kt p) f -> p kt f", p=P))

    # ---- load XT[:, et, tok] = X^T[et*128 + p, tok] (host pre-transposed,
    # so this is a plain fast DMA); chunked by token so projection chunk c
    # starts as soon as its quarter lands; chunks alternate between the two
    # HWDGE queues so they transfer in parallel.
    xt = xtp.tile([P, KT, L], MM_DT, tag="xt")
    for c in range(NC512):
        eng = nc.sync if c % 2 == 0 else nc.scalar
        eng.dma_start(
            xt[:, :, ts(c, 512)],
            x_d[:, ts(c, 512)].rearrange("(et p) t -> p et t", p=P),
        )

    bias_sb = const.tile([P, HPC], F32, tag="bias")
    nc.scalar.dma_start(bias_sb[:], b_d[:])

    # PE warm-up: dummy matmuls with no input deps keep the PE busy while
    # the first DMAs land, so HAM unthrottles (1.2 -> 2.4 GHz) before the
    # real projection starts.
    warm = const.tile([P, 512], MM_DT, tag="warm")
    nc.gpsimd.memset(warm[:], 0.0)
    for _ in range(18):
        pw = psum.tile([P, 512], F32, tag="scores", bufs=2)
        nc.tensor.matmul(pw[:], warm[:, 0:P], warm[:], start=True, stop=True)

    # w columns are host-reordered: block 2*pair   = [Q_h0 | Q_h1] (128 feats)
    #                               block 2*pair+1 = [K_h0 | K_h1]
    def proj_pair(pair, fill=False):
        qt = qkp.tile([P, L], MM_DT, tag="qt")  # 0:64 = Q^T h0, 64:128 = Q^T h1
        kt_t = qkp.tile([P, L], MM_DT, tag="kt")
        # kt first: scores q-tile 0 needs ALL of kt but only chunk 0 of qt
        for dst, blk in ((kt_t, 2 * pair + 1), (qt, 2 * pair)):
            pp = psum.tile([P, L], F32, tag="scores", bufs=2)
            for c in range(NC512):
                for k in range(KT):
                    nc.tensor.matmul(
                        pp[:, ts(c, 512)],
                        w_sb[:, k, ts(blk, P)],
                        xt[:, k, ts(c, 512)],
                        start=(k == 0),
                        stop=(k == KT - 1),
                    )
                nc.vector.tensor_scalar_add(
                    dst[:, ts(c, 512)], pp[:, ts(c, 512)], bias_sb[:, blk : blk + 1]
                )
                if fill and blk == 2 * pair + 1:
                    # keep the PE's activity monitor warm while the next xt
                    # chunk is still in flight (idle >3.4us re-throttles);
                    # reuse the already-consumed psum chunk as scratch
                    for _ in range(6):
                        nc.tensor.matmul(
                            pp[:, ts(c, 512)], warm[:, 0:P], warm[:],
                            start=True, stop=True,
                        )
        return qt, kt_t

    def scores_head(qt, kt_t, h, off):
        for q in range(NQ):
            ps = psum.tile([P, L], F32, tag="scores", bufs=2)
            for c in range(NC512):
                nc.tensor.matmul(
                    ps[:, ts(c, 512)],
                    qt[off : off + DKV, ts(q, P)],
                    kt_t[off : off + DKV, ts(c, 512)],
                    start=True,
                    stop=True,
                )
            ex = expp.tile([P, L], F32, tag="exp")
            rsum = smalls.tile([P, 1], F32, tag="rsum")
            nc.scalar.activation(
                ex[:],
                ps[:],
                mybir.ActivationFunctionType.Exp,
                scale=1.0 / np.sqrt(DKV),
                accum_out=rsum[:],
            )
            rinv = smalls.tile([P, 1], F32, tag="rinv")
            nc.vector.reciprocal(rinv[:], rsum[:])
            ot = outp.tile([P, L], F32, tag="outp")
            nc.vector.tensor_scalar_mul(ot[:], ex[:], rinv[:])
            nc.sync.dma_start(out_d[h, ts(q, P), :], ot[:])

    # proj pair0 -> scores h0/h1 (output DMA starts early) -> proj pair1
    # -> scores h2/h3; proj pair1's PE work hides inside scores h0/h1.
    qt0, kt0 = proj_pair(0, fill=True)
    scores_head(qt0, kt0, 0, 0)
    qt1, kt1 = proj_pair(1)
    scores_head(qt0, kt0, 1, DKV)
    scores_head(qt1, kt1, 2, 0)
    scores_head(qt1, kt1, 3, DKV)


def build():
    global _cached_nc
    if _cached_nc is not None:
        return _cached_nc
    nc = bacc.Bacc("TRN2", target_bir_lowering=False, debug=False)
    with tile.TileContext(nc) as tc, ExitStack() as ctx:
        _emit(tc, ctx)
    nc.compile()
    _cached_nc = nc
    return nc


def _shard_inputs(X, W_qkv, b_qkv):
    X = np.ascontiguousarray(np.asarray(X, dtype=np.float32))
    W = np.asarray(W_qkv, dtype=np.float32)
    bq = np.asarray(b_qkv, dtype=np.float32)
    in_maps = []
    for core in range(N_CORES):
        b = core // 4
        g = core % 4
        heads = list(range(g * HPC, (g + 1) * HPC))
        # per head h: W cols [h*3*DKV, h*3*DKV+DKV) = Q feats,
        #             [h*3*DKV+DKV, h*3*DKV+2*DKV) = K feats.
        # Reorder into per-pair stacked blocks: [Q_h0|Q_h1], [K_h0|K_h1], ...
        wq = [W[:, h * 3 * DKV : h * 3 * DKV + DKV] for h in heads]
        wk = [W[:, h * 3 * DKV + DKV : h * 3 * DKV + 2 * DKV] for h in heads]
        bqh = [bq[h * 3 * DKV : h * 3 * DKV + DKV] for h in heads]
        bkh = [bq[h * 3 * DKV + DKV : h * 3 * DKV + 2 * DKV] for h in heads]
        w_blocks, b_blocks = [], []
        for pair in range(HPC // 2):
            w_blocks += [wq[2 * pair], wq[2 * pair + 1]]
            w_blocks += [wk[2 * pair], wk[2 * pair + 1]]
            b_blocks += [np.concatenate([bqh[2 * pair], bqh[2 * pair + 1]])]
            b_blocks += [np.concatenate([bkh[2 * pair], bkh[2 * pair + 1]])]
        mm_np = mybir.dt.np(MM_DT)
        w_sel = np.concatenate(w_blocks, axis=1)
        b_sel = np.stack(b_blocks, axis=1)
        in_maps.append(
            {
                "x": np.ascontiguousarray(X[b].T).astype(mm_np),
                "w": np.ascontiguousarray(w_sel).astype(mm_np),
                "bqk": np.ascontiguousarray(b_sel),
            }
        )
    return in_maps


def kernel(X, W_qkv, b_qkv):
    nc = build()
    in_maps = _shard_inputs(X, W_qkv, b_qkv)
    res = run_bass_kernel_spmd(nc, in_maps, core_ids=list(range(N_CORES)), trace=TRACE)
    out = np.empty((B, H, L, L), dtype=np.float32)
    for core in range(N_CORES):
        b = core // 4
        g = core % 4
        out[b, g * HPC : (g + 1) * HPC] = res.results[core]["out"]
    kernel.last_results = res
    return out
